# revision 1
# baseline (speedup 1.0000x reference)
"""Distributed causal multi-head attention (Bass/Tile, 8 TRN2 NeuronCores).

Sharding: core = (batch b, rank r) with b = core // 4, r = core % 4.
Within a batch group of 4 cores, rank r owns query rows {g : g % 4 == r}
(row-interleaved sequence parallelism).  Every core runs the IDENTICAL
graph; rank-dependence lives entirely in the input data (x^T shard and a
small diagonal-mask tensor built on the host).

Per core:
  q^T, k^T = (x_own @ Wq/Wk)^T   [C, 512]   (transposed orientation)
  v        =  x_own @ Wv         [512, C]   (normal orientation)
  AllGather (bf16) of packed [k^T | v] across the 4 ranks
  scores[tq, tk] = q^T.T @ k^T chunks  (keys in rank-permuted order)
  softmax: fused exp + row-sum via activation(accum_out), per-partition
  normalize, PE-transpose att tiles, AV matmul (2 heads col-packed)
  producing out^T directly, then y = out^T.T @ Wo.
"""

import numpy as np

B, T, C, H = 2, 2048, 1024, 16
D = C // H            # 64
R = 4                 # ranks per batch group
TOWN = T // R         # 512 rows owned per core
NJ = T // 512         # 4 key 512-chunks
NT = TOWN // 128      # 4 local query 128-tiles
CC = C // 128         # 8 contraction chunks
PAIRS = H // 2        # 8 head pairs
KT_ELEMS = C * TOWN   # k^T shard elems
V_ELEMS = TOWN * C    # v shard elems
AG_ELEMS = KT_ELEMS + V_ELEMS
SCALE = 1.0 / 32.0    # 1/sqrt(C)
NEG = -1e30

_cached_nc = None
last_result = None
_DEBUG = False


def _dbg(nc, P, col, ap, width):
    if P.get("dbg_ext") is not None:
        nc.sync.dma_start(P["dbg_ext"][:, col : col + width], ap)


def _qkv_phase(nc, P, mybir):
    """k^T and v in halves, four quarter-size AllGathers interleaved, then q^T."""
    F32, BF16 = mybir.dt.float32, mybir.dt.bfloat16
    wqkv_sb, xt_sb = P["wqkv_sb"], P["xt_sb"]
    mm_p = P["mm_p"]
    groups = [[0, 1, 2, 3], [4, 5, 6, 7]]
    VW = PAIRS * 130  # 1040
    HKT = 4 * TOWN * 128  # elems per half k^T bounce

    kt_loc = P["kv_p"].tile([128, CC * TOWN], BF16, tag="kt_loc")
    v_loc = P["kv_p"].tile([128, NT * VW], BF16, tag="v_loc")
    nc.vector.memset(
        v_loc[:].rearrange("p (ch x) -> p ch x", x=65)[:, :, 64:65], 1.0
    )

    def kt_half(half):
        for kc in range(4 * half, 4 * (half + 1)):
            ps = mm_p.tile([128, 512], F32, tag="sc")
            for cc in range(CC):
                nc.tensor.matmul(
                    ps[:],
                    wqkv_sb[:, cc * 3 * C + C + kc * 128 : cc * 3 * C + C + (kc + 1) * 128],
                    xt_sb[:, cc * TOWN : (cc + 1) * TOWN],
                    start=(cc == 0),
                    stop=(cc == CC - 1),
                )
            nc.vector.tensor_copy(kt_loc[:, kc * TOWN : (kc + 1) * TOWN], ps[:])
        bounce = P["dram_p"].tile([HKT], BF16, tag=f"bounce_kt{half}")
        for i, kc in enumerate(range(4 * half, 4 * (half + 1))):
            nc.sync.dma_start(
                bounce[i * TOWN * 128 : (i + 1) * TOWN * 128].rearrange("(p f) -> p f", p=128),
                kt_loc[:, kc * TOWN : (kc + 1) * TOWN],
            )
        gath = P["dram_p"].tile([R * HKT], BF16, tag=f"gathered_kt{half}")
        nc.gpsimd.collective_compute(
            "AllGather", mybir.AluOpType.bypass, replica_groups=groups,
            ins=[bounce.opt()], outs=[gath.opt()],
        )
        P[f"gathered_kt{half}"] = gath

    def v_half(hf):
        for t in range(NT):
            ps = mm_p.tile([128, 512], F32, tag="sc")
            for cc in range(CC):
                nc.tensor.matmul(
                    ps[:],
                    xt_sb[:, cc * TOWN + t * 128 : cc * TOWN + (t + 1) * 128],
                    wqkv_sb[:, cc * 3 * C + 2 * C + hf * 512 : cc * 3 * C + 2 * C + (hf + 1) * 512],
                    start=(cc == 0),
                    stop=(cc == CC - 1),
                )
            for hh in range(2):
                nc.vector.tensor_copy(
                    v_loc[:, t * VW + 4 * hf * 130 : t * VW + 4 * (hf + 1) * 130].rearrange(
                        "p (pr x) -> p pr x", x=130
                    )[:, :, hh * 65 : hh * 65 + 64],
                    ps[:].rearrange("p (pr hc) -> p pr hc", hc=128)[:, :, hh * 64 : (hh + 1) * 64],
                )
        bounce = P["dram_p"].tile([NT * 128 * 520], BF16, tag=f"bounce_v{hf}")
        for t in range(NT):
            nc.sync.dma_start(
                bounce[t * 520 * 128 : (t + 1) * 520 * 128].rearrange("(p f) -> p f", p=128),
                v_loc[:, t * VW + 4 * hf * 130 : t * VW + 4 * (hf + 1) * 130],
            )
        gath = P["dram_p"].tile([R * NT * 128 * 520], BF16, tag=f"gathered_v{hf}")
        nc.gpsimd.collective_compute(
            "AllGather", mybir.AluOpType.bypass, replica_groups=groups,
            ins=[bounce.opt()], outs=[gath.opt()],
        )
        P[f"gathered_v{hf}"] = gath

    kt_half(0)
    v_half(0)
    kt_half(1)
    v_half(1)

    qt_sb = P["qt_p"].tile([128, CC * TOWN], BF16, tag="qt")
    for qc in range(CC):
        ps = mm_p.tile([128, 512], F32, tag="sc")
        for cc in range(CC):
            nc.tensor.matmul(
                ps[:],
                wqkv_sb[:, cc * 3 * C + qc * 128 : cc * 3 * C + (qc + 1) * 128],
                xt_sb[:, cc * TOWN : (cc + 1) * TOWN],
                start=(cc == 0),
                stop=(cc == CC - 1),
            )
        nc.vector.tensor_scalar_mul(qt_sb[:, qc * TOWN : (qc + 1) * TOWN], ps[:], SCALE)
    P["qt_sb"] = qt_sb


def _gather_kv_pair(nc, P, p, mybir):
    """Load this head pair's gathered k^T and v into SBUF (rank-major cols)."""
    BF16 = mybir.dt.bfloat16
    half, pl = p // 4, p % 4
    gkt = P[f"gathered_kt{half}"]
    gv = P[f"gathered_v{half}"]
    HKT = 4 * TOWN * 128
    ktg = P["ktg_p"].tile([128, 2048], BF16, tag="ktg")
    for s in range(R):
        src = gkt[
            s * HKT + pl * 128 * TOWN : s * HKT + (pl + 1) * 128 * TOWN
        ].rearrange("(q f) -> q f", q=128)
        nc.sync.dma_start(ktg[:, s * 512 : (s + 1) * 512], src)
    vg = P["vg_p"].tile([128, 2080], BF16, tag="vg")
    for s in range(R):
        src = gv[s * NT * 128 * 520 : (s + 1) * NT * 128 * 520].rearrange(
            "(j i x) -> i j x", i=128, x=520
        )[:, :, pl * 130 : (pl + 1) * 130]
        nc.sync.dma_start(
            vg[:, s * 520 : (s + 1) * 520].rearrange("i (j x) -> i j x", x=130), src
        )
    return ktg, vg


def _attention_pair2(nc, P, pA, ktgA, vgA, pB, ktgB, vgB, mybir):
    """Scores^T + exp + AV for two head pairs, j-loops interleaved."""
    F32, BF16 = mybir.dt.float32, mybir.dt.bfloat16
    AFT = mybir.ActivationFunctionType
    qt_sb, dmask = P["qt_sb"], P["dmask"]
    mm_p, attT_p, sm_p = P["mm_p"], P["attT_p"], P["sm_p"]

    avsP = {}
    for p in (pA, pB):
        a0 = P["av_p"].tile([65, TOWN], F32, tag="av")
        a1 = P["av_p"].tile([65, TOWN], F32, tag="av")
        avsP[p] = [a0, a1]
    first = {pA: [True, True], pB: [True, True]}
    for j in range(16):
        jj, sb = j // 4, j % 4
        l0 = jj * 128
        kcol = sb * 512 + jj * 128
        vcol = (sb * 4 + jj) * 130
        for p, ktg, vg in ((pA, ktgA, vgA), (pB, ktgB, vgB)):
            avs = avsP[p]
            for hh in range(2):
                ps = mm_p.tile([128, 512], F32, tag="sc")
                nc.tensor.matmul(
                    ps[:, l0:],
                    ktg[hh * 64 : (hh + 1) * 64, kcol : kcol + 128],
                    qt_sb[hh * 64 : (hh + 1) * 64, p * TOWN + l0 : (p + 1) * TOWN],
                    start=True,
                    stop=True,
                )
                nc.vector.tensor_add(
                    ps[:, l0 : l0 + 128],
                    ps[:, l0 : l0 + 128],
                    dmask[:, sb * 256 + hh * 128 : sb * 256 + (hh + 1) * 128],
                )
                att2 = attT_p.tile([128, 512], BF16, tag="attT")
                nc.scalar.activation(att2[:, l0:], ps[:, l0:], AFT.Exp)
                nc.tensor.matmul(
                    avs[hh][:, l0:],
                    vg[:, vcol + hh * 65 : vcol + (hh + 1) * 65],
                    att2[:, l0:],
                    start=first[p][hh],
                    stop=(j == 15),
                )
                first[p][hh] = False

    # unnormalized out^T and denominator rows; normalization is deferred
    for p in (pA, pB):
        avs = avsP[p]
        den_st = P["sm_p"].tile([128, 2 * TOWN], F32, tag="den_st")
        for hh in range(2):
            nc.vector.tensor_copy(
                P["outT_sb"][hh * 64 : (hh + 1) * 64, p * TOWN : (p + 1) * TOWN],
                avs[hh][0:64, :],
            )
            nc.vector.tensor_copy(
                den_st[64:65, hh * TOWN : (hh + 1) * TOWN], avs[hh][64:65, :]
            )
        dr = 32 * (p // 2) + 2 * (p % 2)
        nc.sync.dma_start(P["den_mat"][dr : dr + 2, :], den_st[64:65, :])


def _normalize_pg(nc, P, pg, mybir):
    """Normalize the four heads of one pair-group (overlaps later groups)."""
    F32, BF16 = mybir.dt.float32, mybir.dt.bfloat16
    den_mat, outT_sb = P["den_mat"], P["outT_sb"]
    r0 = 32 * pg
    nc.vector.reciprocal(den_mat[r0 : r0 + 4, :], den_mat[r0 : r0 + 4, :])
    recb4 = P["sm_p"].tile([128, TOWN], BF16, tag="recb4")
    nc.vector.tensor_copy(recb4[r0 : r0 + 4, :], den_mat[r0 : r0 + 4, :])
    for hi in range(4):
        h = 4 * pg + hi
        lo = (h % 2) * 64
        recb = P["sm_p"].tile([1, TOWN], BF16, tag="recb")
        nc.sync.dma_start(recb[:], recb4[r0 + hi : r0 + hi + 1, :])
        bc = P["av_p"].tile([128, TOWN], F32, tag="av")
        nc.tensor.matmul(
            bc[lo : lo + 64, :], P["ones64"][:], recb[:],
            start=True, stop=True, tile_position=(0, lo),
        )
        bcs = P["sm_p"].tile([128, TOWN], BF16, tag="bcs")
        nc.vector.tensor_copy(bcs[lo : lo + 64, :], bc[lo : lo + 64, :])
        nc.vector.tensor_mul(
            outT_sb[lo : lo + 64, (h // 2) * TOWN : (h // 2 + 1) * TOWN],
            outT_sb[lo : lo + 64, (h // 2) * TOWN : (h // 2 + 1) * TOWN],
            bcs[lo : lo + 64, :],
        )


def _wo_phase(nc, P, mybir):
    from concourse.bass import ts
    F32 = mybir.dt.float32
    BF16 = mybir.dt.bfloat16
    wo_sb = P["w_p"].tile([128, CC * C], BF16, tag="wo")
    for cc in range(CC):
        nc.sync.dma_start(wo_sb[:, cc * C : (cc + 1) * C], P["wo_ext"][ts(cc, 128), :])
    outT_sb, mm_p = P["outT_sb"], P["mm_p"]
    y_sb = P["y_p"].tile([128, NT * C], F32, tag="y")
    for t in range(NT):
        for hf in range(2):
            ps = mm_p.tile([128, 512], F32, tag="sc")
            for cc in range(CC):
                nc.tensor.matmul(
                    ps[:],
                    outT_sb[:, cc * TOWN + t * 128 : cc * TOWN + (t + 1) * 128],
                    wo_sb[:, cc * C + hf * 512 : cc * C + (hf + 1) * 512],
                    start=(cc == 0),
                    stop=(cc == CC - 1),
                )
            nc.vector.tensor_copy(y_sb[:, t * C + hf * 512 : t * C + (hf + 1) * 512], ps[:])
    for t in range(NT):
        nc.sync.dma_start(P["out_ext"][t * 128 : (t + 1) * 128, :], y_sb[:, t * C : (t + 1) * C])


def _body(nc, P, mybir):
    from concourse.bass import ts

    F32, BF16 = mybir.dt.float32, mybir.dt.bfloat16

    ones64 = P["const_p"].tile([1, 64], BF16, tag="ones64")
    nc.vector.memset(ones64[:], 1.0)
    P["ones64"] = ones64
    dmask = P["const_p"].tile([128, 1024], F32, tag="dmask")
    nc.sync.dma_start(dmask[:], P["dmask_ext"][:])
    P["dmask"] = dmask

    xt_sb = P["x_p"].tile([128, CC * TOWN], BF16, tag="xt")
    for cc in range(CC):
        nc.sync.dma_start(xt_sb[:, cc * TOWN : (cc + 1) * TOWN], P["xt_ext"][ts(cc, 128), :])
    P["xt_sb"] = xt_sb
    wqkv_sb = P["w_p"].tile([128, CC * 3 * C], BF16, tag="wqkv")
    for part in (1, 2, 0):  # k first (feeds the AllGather), then v, then q
        for cc in range(CC):
            nc.sync.dma_start(
                wqkv_sb[:, cc * 3 * C + part * C : cc * 3 * C + (part + 1) * C],
                P["wqkv_ext"][ts(cc, 128), part * C : (part + 1) * C],
            )
    P["wqkv_sb"] = wqkv_sb

    _qkv_phase(nc, P, mybir)

    outT_sb = P["outT_p"].tile([128, PAIRS * TOWN], BF16, tag="outT")
    P["outT_sb"] = outT_sb
    den_mat = P["sm_p"].tile([128, TOWN], F32, tag="den_mat")
    P["den_mat"] = den_mat
    for pg in range(PAIRS // 2):
        pA, pB = 2 * pg, 2 * pg + 1
        ktgA, vgA = _gather_kv_pair(nc, P, pA, mybir)
        ktgB, vgB = _gather_kv_pair(nc, P, pB, mybir)
        _attention_pair2(nc, P, pA, ktgA, vgA, pB, ktgB, vgB, mybir)
        _normalize_pg(nc, P, pg, mybir)

    _wo_phase(nc, P, mybir)


def _build():
    import concourse.mybir as mybir
    import concourse.tile as tile
    from concourse import bacc

    F32, BF16 = mybir.dt.float32, mybir.dt.bfloat16

    nc = bacc.Bacc("TRN2", target_bir_lowering=False, debug=False, num_devices=8)
    P = {
        "xt_ext": nc.declare_dram_parameter("xt", [C, TOWN], BF16, isOutput=False),
        "wqkv_ext": nc.declare_dram_parameter("wqkv", [C, 3 * C], BF16, isOutput=False),
        "wo_ext": nc.declare_dram_parameter("wo", [C, C], BF16, isOutput=False),
        "dmask_ext": nc.declare_dram_parameter("dmask", [128, 1024], F32, isOutput=False),
        "out_ext": nc.declare_dram_parameter("out", [TOWN, C], F32, isOutput=True),
    }
    if _DEBUG:
        P["dbg_ext"] = nc.declare_dram_parameter("dbg", [128, 10240], BF16, isOutput=True)

    with tile.TileContext(nc) as tc:
        with (
            tc.tile_pool(name="const", bufs=1) as const_p,
            tc.tile_pool(name="w", bufs=1) as w_p,
            tc.tile_pool(name="x", bufs=1) as x_p,
            tc.tile_pool(name="qt", bufs=1) as qt_p,
            tc.tile_pool(name="kv", bufs=1) as kv_p,
            tc.tile_pool(name="ktg", bufs=4) as ktg_p,
            tc.tile_pool(name="vg", bufs=4) as vg_p,
            tc.tile_pool(name="attT", bufs=8) as attT_p,
            tc.tile_pool(name="outT", bufs=1) as outT_p,
            tc.tile_pool(name="y", bufs=1) as y_p,
            tc.tile_pool(name="sm", bufs=2) as sm_p,
            tc.tile_pool(name="mmp", bufs=4, space="PSUM") as mm_p,
            tc.tile_pool(name="avp", bufs=4, space="PSUM") as av_p,
            tc.tile_pool(name="dram", bufs=1, space="DRAM") as dram_p,
        ):
            P.update(
                const_p=const_p, w_p=w_p, x_p=x_p, qt_p=qt_p, kv_p=kv_p,
                ktg_p=ktg_p, vg_p=vg_p, attT_p=attT_p,
                outT_p=outT_p, y_p=y_p, sm_p=sm_p, mm_p=mm_p, av_p=av_p,
                dram_p=dram_p,
            )
            _body(nc, P, mybir)

    nc.finalize()
    return nc


def kernel(x, Wqkv, bqkv, Wo, bo):
    global _cached_nc, last_result
    import ml_dtypes
    from concourse.bass_utils import run_bass_kernel_spmd

    if _cached_nc is None:
        _cached_nc = _build()
    nc = _cached_nc

    bf16 = ml_dtypes.bfloat16
    x = np.asarray(x, dtype=np.float32)
    wq_b = np.ascontiguousarray(np.asarray(Wqkv, dtype=np.float32).astype(bf16))
    wo_b = np.ascontiguousarray(np.asarray(Wo, dtype=np.float32).astype(bf16))

    # transposed diagonal-chunk causal mask: partition = key i, free = (s, query p)
    i_idx = np.arange(128)[:, None, None]
    s_idx = np.arange(R)[None, :, None]
    p_idx = np.arange(128)[None, None, :]

    in_maps = []
    for core in range(8):
        b, r = divmod(core, R)
        xt = np.ascontiguousarray(x[b].T[:, r::R].astype(bf16))
        masked = (i_idx > p_idx) | ((i_idx == p_idx) & (s_idx > r))
        dm = np.where(masked, np.float32(NEG), np.float32(0.0)).reshape(128, 4, 128)
        dm = np.repeat(dm, 2, axis=1).reshape(128, 1024)
        in_maps.append(
            {"xt": xt, "wqkv": wq_b, "wo": wo_b, "dmask": np.ascontiguousarray(dm)}
        )

    last_result = run_bass_kernel_spmd(nc, in_maps, core_ids=list(range(8)))

    y = np.empty((B, T, C), dtype=np.float32)
    for core in range(8):
        b, r = divmod(core, R)
        y[b, r::R, :] = last_result.results[core]["out"]
    return y



# revision 13
# speedup vs baseline: 1.0658x; 1.0658x over previous
"""Distributed causal multi-head attention (Bass/Tile, 8 TRN2 NeuronCores).

Sharding: core = (batch b, head-group g) with b = core // 4, g = core % 4.
Each core owns 4 heads (two pairs) of batch b and computes their QKV from
the full x[b] locally -- no K/V collective at all.  After attention, one
small AllToAll per head pair redistributes the attention outputs from
head-sharded to row-sharded, and each core applies the full Wo to its
512-row block.

Per core:
  q^T, k^T = (x @ Wq/Wk)^T  [128, 2048] per pair  (partition = head dims)
  v        =  x @ Wv        [128, 65] per (head, kchunk), ones col appended
  scores^T = k^T.T @ q^T    row-tiled pairs run concurrently on the PE
  softmax: full-width exp (scalar engine), causal mask as 0/1 post-mult,
  AV matmul with ones column -> unnormalized out^T + denominator row,
  normalize via reciprocal + PE broadcast + fused DVE multiply,
  AllToAll (bf16) across the 4 head-group ranks, y = out^T.T @ Wo.
"""

import numpy as np

B, T, C, H = 2, 2048, 1024, 16
D = C // H            # 64
G = 4                 # head-group ranks per batch
HPC = H // G          # 4 heads per core
NP = HPC // 2         # 2 head pairs per core
TOWN = T // G         # 512 output rows owned per core
NQG = T // 512        # 4 query groups of 512
NKC = T // 128        # 16 key chunks of 128
CC = C // 128         # 8 contraction chunks
WCOL = 3 * HPC * D    # 768 packed qkv columns per core
SCALE = 1.0 / 32.0    # 1/sqrt(C), folded into Wq on host

_cached_nc = None
last_result = None


def _load_phase(nc, P, mybir):
    from concourse.bass import ts

    F32, BF16 = mybir.dt.float32, mybir.dt.bfloat16

    mask = P["const_p"].tile([128, 128], BF16, tag="mask")
    nc.sync.dma_start(mask[:], P["mask_ext"][:])
    P["mask"] = mask
    ones = P["const_p"].tile([128, 64], BF16, tag="ones")
    nc.vector.memset(ones[:], 1.0)
    P["ones"] = ones

    xt_sb = P["x_p"].tile([128, CC * T], BF16, tag="xt")
    for cc in range(CC):
        nc.sync.dma_start(xt_sb[:, cc * T : (cc + 1) * T], P["xt_ext"][ts(cc, 128), :])
    P["xt_sb"] = xt_sb
    wqkv_sb = P["w_p"].tile([128, CC * WCOL], BF16, tag="wqkv")
    for cc in range(CC):
        nc.sync.dma_start(
            wqkv_sb[:, cc * WCOL : (cc + 1) * WCOL], P["wqkv_ext"][ts(cc, 128), :]
        )
    P["wqkv_sb"] = wqkv_sb
    wo_sb = P["w_p"].tile([128, CC * C], BF16, tag="wo")
    for cc in range(CC):
        nc.sync.dma_start(wo_sb[:, cc * C : (cc + 1) * C], P["wo_ext"][ts(cc, 128), :])
    P["wo_sb"] = wo_sb


def _qkv_phase(nc, P, mybir):
    """q^T, k^T per pair [128, T]; v per (head, kchunk) [128, 65] with ones."""
    F32, BF16 = mybir.dt.float32, mybir.dt.bfloat16
    AFT = mybir.ActivationFunctionType
    xt_sb, wqkv_sb = P["xt_sb"], P["wqkv_sb"]

    qt, kt = [], []
    for p in range(NP):
        qt.append(P["qk_p"].tile([128, T], BF16, tag=f"qt{p}", name=f"qt{p}"))
        kt.append(P["qk_p"].tile([128, T], BF16, tag=f"kt{p}", name=f"kt{p}"))
    P["qt"], P["kt"] = qt, kt

    # v_sb: per (head, kchunk) a [128, 65] block, col 64 = 1.0 (denominator)
    v_sb = P["v_p"].tile([128, HPC * NKC * 65], BF16, tag="v")
    nc.vector.memset(
        v_sb[:].rearrange("p (hj x) -> p hj x", x=65)[:, :, 64:65], 1.0
    )
    P["v_sb"] = v_sb

    # q^T and k^T: stationary = Wqkv column block, moving = x^T
    for p in range(NP):
        for kind, dst in ((0, qt[p]), (1, kt[p])):
            mcol = (kind * NP + p) * 128
            for tb in range(2):  # two 1024-wide t blocks
                ps = P["mm_p"].tile([128, 1024], F32, tag="mm")
                for nh in range(2):
                    t0 = tb * 1024 + nh * 512
                    for cc in range(CC):
                        nc.tensor.matmul(
                            ps[:, nh * 512 : (nh + 1) * 512],
                            wqkv_sb[:, cc * WCOL + mcol : cc * WCOL + mcol + 128],
                            xt_sb[:, cc * T + t0 : cc * T + t0 + 512],
                            start=(cc == 0),
                            stop=(cc == CC - 1),
                        )
                nc.scalar.activation(
                    dst[:, tb * 1024 : (tb + 1) * 1024], ps[:], AFT.Copy
                )

    # v: stationary = x^T chunk, moving = Wv columns -> [tchunk, 4*64]
    for j in range(NKC):
        ps = P["av_p"].tile([128, 512], F32, tag="av")
        for cc in range(CC):
            nc.tensor.matmul(
                ps[:, 0:256],
                xt_sb[:, cc * T + j * 128 : cc * T + (j + 1) * 128],
                wqkv_sb[:, cc * WCOL + 512 : cc * WCOL + 768],
                start=(cc == 0),
                stop=(cc == CC - 1),
            )
        nc.vector.tensor_copy(
            v_sb[:].rearrange("p (hj x) -> p hj x", x=65)[:, j::NKC, 0:64],
            ps[:, 0:256].rearrange("p (h d) -> p h d", d=64),
        )


def _attention_pair(nc, P, p, mybir):
    """Scores^T + exp + AV + normalize for one head pair, all query groups."""
    F32, BF16 = mybir.dt.float32, mybir.dt.bfloat16
    AFT = mybir.ActivationFunctionType
    qt, kt, v_sb, mask = P["qt"][p], P["kt"][p], P["v_sb"], P["mask"]
    outT = [
        P["outT_p"].tile([64, T], BF16, tag=f"outT{p}{hh}", name=f"outT{p}{hh}")
        for hh in range(2)
    ]
    P[f"outT{p}"] = outT

    for qg in range(NQG):
        njc = 4 * qg + 4          # key chunks (incl. diagonal) for this block
        avs = [
            P["av_p"].tile([65, 512], F32, tag="av", name=f"av{hh}")
            for hh in range(2)
        ]
        for jp in range(njc // 2):
            att2 = [None, None]
            for hh in range(2):
                ps = P["mm_p"].tile([128, 1024], F32, tag="mm")
                for dj in range(2):
                    j = 2 * jp + dj
                    nc.tensor.matmul(
                        ps[:, dj * 512 : (dj + 1) * 512],
                        kt[hh * 64 : (hh + 1) * 64, j * 128 : (j + 1) * 128],
                        qt[hh * 64 : (hh + 1) * 64, qg * 512 : (qg + 1) * 512],
                        start=True,
                        stop=True,
                        tile_position=(hh * 64, 0),
                    )
                a2 = P["att_p"].tile([128, 1024], BF16, tag="att")
                nc.scalar.activation(a2[:], ps[:], AFT.Exp)
                att2[hh] = a2
            for hh in range(2):
                h = 2 * p + hh
                for dj in range(2):
                    j = 2 * jp + dj
                    l0 = (j - 4 * qg) * 128  # first valid query col (diag)
                    lo = max(l0, 0)
                    if l0 >= 0:  # diagonal chunk: triangular 0/1 mask
                        nc.vector.tensor_mul(
                            att2[hh][:, dj * 512 + l0 : dj * 512 + l0 + 128],
                            att2[hh][:, dj * 512 + l0 : dj * 512 + l0 + 128],
                            mask[:],
                        )
                    nc.tensor.matmul(
                        avs[hh][:, lo:],
                        v_sb[:, (h * NKC + j) * 65 : (h * NKC + j) * 65 + 65],
                        att2[hh][:, dj * 512 + lo : (dj + 1) * 512],
                        start=(j == 0),
                        stop=(j == njc - 1),
                    )
        # normalize: recip(den row) -> PE broadcast -> fused multiply
        for hh in range(2):
            recb = P["sm_p"].tile([128, 512], BF16, tag="recb")
            with nc.allow_low_precision(reason="bf16 softmax denominator"):
                nc.vector.reciprocal(recb[64:65, :], avs[hh][64:65, :])
            bc = P["bc_p"].tile([128, 512], F32, tag="bc")
            nc.tensor.matmul(
                bc[0:64, :], P["ones"][64:65, :], recb[64:65, :],
                start=True, stop=True, tile_position=(64, 0),
            )
            dst = outT[hh][:, qg * 512 : (qg + 1) * 512]
            nc.vector.tensor_copy(dst, avs[hh][0:64, :])
            nc.vector.tensor_mul(dst, dst, bc[0:64, :])


def _a2a_pair(nc, P, p, mybir):
    """Bounce out^T to DRAM, 8-way AllToAll (4-way exchange, dup-written).

    AllToAll only supports the full 8-core mesh, so each core writes its
    row-block r into chunks r AND 4+r; the cross-batch copy is ignored on
    the receive side (cond-gated reads in _wo_phase).
    """
    BF16 = mybir.dt.bfloat16
    outT = P[f"outT{p}"]
    groups = [[0, 1, 2, 3, 4, 5, 6, 7]]
    CH = 128 * 512  # elems per rank chunk
    bounce = P["dram_p"].tile([2 * G * CH], BF16, tag=f"bounce{p}")
    for r in range(G):
        for hh in range(2):
            for half in range(2):
                c = half * G + r
                nc.sync.dma_start(
                    bounce[
                        c * CH + hh * 64 * 512 : c * CH + (hh + 1) * 64 * 512
                    ].rearrange("(q f) -> q f", q=64),
                    outT[hh][:, r * 512 : (r + 1) * 512],
                )
    gath = P["dram_p"].tile([2 * G * CH], BF16, tag=f"gath{p}")
    nc.gpsimd.collective_compute(
        "AllToAll", mybir.AluOpType.bypass, replica_groups=groups,
        ins=[bounce.opt()], outs=[gath.opt()],
    )
    P[f"gath{p}"] = gath


def _wo_phase(nc, P, mybir):
    """Load gathered out^T chunks, y = out^T.T @ Wo, stream to DRAM."""
    F32, BF16 = mybir.dt.float32, mybir.dt.bfloat16
    CH = 128 * 512
    pid = nc.sync.partition_id()
    gat = P["gat_p"].tile([128, CC * 512], BF16, tag="gat")
    for r in range(G):
        for p in range(NP):
            cc = 2 * r + p
            for half, cond in ((0, pid < G), (1, pid >= G)):
                c = half * G + r
                src = P[f"gath{p}"][c * CH : (c + 1) * CH].rearrange(
                    "(q f) -> q f", q=128
                )
                nc.sync.dma_start(
                    gat[:, cc * 512 : (cc + 1) * 512], src, cond=cond
                )
    wo_sb = P["wo_sb"]
    AFT = mybir.ActivationFunctionType
    for t in range(4):
        for hf in range(2):
            ps = P["mm_p"].tile([128, 1024], F32, tag="mm")
            for cc in range(CC):
                nc.tensor.matmul(
                    ps[:, 0:512],
                    gat[:, cc * 512 + t * 128 : cc * 512 + (t + 1) * 128],
                    wo_sb[:, cc * C + hf * 512 : cc * C + (hf + 1) * 512],
                    start=(cc == 0),
                    stop=(cc == CC - 1),
                )
            y_sb = P["y_p"].tile([128, 512], F32, tag="y")
            nc.scalar.activation(y_sb[:], ps[:, 0:512], AFT.Copy)
            nc.sync.dma_start(
                P["out_ext"][t * 128 : (t + 1) * 128, hf * 512 : (hf + 1) * 512],
                y_sb[:],
            )


def _body(nc, P, mybir):
    _load_phase(nc, P, mybir)
    _qkv_phase(nc, P, mybir)
    for p in range(NP):
        _attention_pair(nc, P, p, mybir)
        _a2a_pair(nc, P, p, mybir)
    _wo_phase(nc, P, mybir)


def _build():
    import concourse.mybir as mybir
    import concourse.tile as tile
    from concourse import bacc

    F32, BF16 = mybir.dt.float32, mybir.dt.bfloat16

    nc = bacc.Bacc("TRN2", target_bir_lowering=False, debug=False, num_devices=8)
    P = {
        "xt_ext": nc.declare_dram_parameter("xt", [C, T], BF16, isOutput=False),
        "wqkv_ext": nc.declare_dram_parameter("wqkv", [C, WCOL], BF16, isOutput=False),
        "wo_ext": nc.declare_dram_parameter("wo", [C, C], BF16, isOutput=False),
        "mask_ext": nc.declare_dram_parameter("mask", [128, 128], BF16, isOutput=False),
        "out_ext": nc.declare_dram_parameter("out", [TOWN, C], F32, isOutput=True),
    }

    with tile.TileContext(nc) as tc:
        with (
            tc.tile_pool(name="const", bufs=1) as const_p,
            tc.tile_pool(name="w", bufs=1) as w_p,
            tc.tile_pool(name="x", bufs=1) as x_p,
            tc.tile_pool(name="qk", bufs=1) as qk_p,
            tc.tile_pool(name="v", bufs=1) as v_p,
            tc.tile_pool(name="att", bufs=4) as att_p,
            tc.tile_pool(name="outT", bufs=1) as outT_p,
            tc.tile_pool(name="gat", bufs=1) as gat_p,
            tc.tile_pool(name="y", bufs=2) as y_p,
            tc.tile_pool(name="sm", bufs=2) as sm_p,
            tc.tile_pool(name="mm", bufs=2, space="PSUM") as mm_p,
            tc.tile_pool(name="av", bufs=2, space="PSUM") as av_p,
            tc.tile_pool(name="bc", bufs=1, space="PSUM") as bc_p,
            tc.tile_pool(name="dram", bufs=1, space="DRAM") as dram_p,
        ):
            P.update(
                const_p=const_p, w_p=w_p, x_p=x_p, qk_p=qk_p, v_p=v_p,
                att_p=att_p, outT_p=outT_p, gat_p=gat_p, y_p=y_p, sm_p=sm_p,
                mm_p=mm_p, av_p=av_p, bc_p=bc_p, dram_p=dram_p,
            )
            _body(nc, P, mybir)

    nc.finalize()
    return nc


def kernel(x, Wqkv, bqkv, Wo, bo):
    global _cached_nc, last_result
    import ml_dtypes
    from concourse.bass_utils import run_bass_kernel_spmd

    if _cached_nc is None:
        _cached_nc = _build()
    nc = _cached_nc

    bf16 = ml_dtypes.bfloat16
    x = np.asarray(x, dtype=np.float32)
    Wqkv = np.asarray(Wqkv, dtype=np.float32)
    wo_b = np.ascontiguousarray(np.asarray(Wo, dtype=np.float32).astype(bf16))

    # lower-triangle 0/1 mask for diagonal blocks: partition = key, free = query
    tri = (np.arange(128)[:, None] <= np.arange(128)[None, :]).astype(bf16)
    tri = np.ascontiguousarray(tri)

    in_maps = []
    for core in range(8):
        b, g = divmod(core, G)
        xt = np.ascontiguousarray(x[b].T.astype(bf16))
        c0 = g * HPC * D
        wq = Wqkv[:, c0 : c0 + HPC * D] * SCALE
        wk = Wqkv[:, C + c0 : C + c0 + HPC * D]
        wv = Wqkv[:, 2 * C + c0 : 2 * C + c0 + HPC * D]
        wqkv = np.ascontiguousarray(
            np.concatenate([wq, wk, wv], axis=1).astype(bf16)
        )
        in_maps.append({"xt": xt, "wqkv": wqkv, "wo": wo_b, "mask": tri})

    last_result = run_bass_kernel_spmd(nc, in_maps, core_ids=list(range(8)))

    y = np.empty((B, T, C), dtype=np.float32)
    for core in range(8):
        b, g = divmod(core, G)
        y[b, g * TOWN : (g + 1) * TOWN, :] = last_result.results[core]["out"]
    return y
